# revision 4
# baseline (speedup 1.0000x reference)
"""AdaptivePatchEmbedding Bass kernel for 8 TRN2 NeuronCores.

Data-parallel: 32768 independent regions (BC=2048 rows x R=16 regions),
4096 regions per core.  Per region: route (2-layer MLP + gumbel argmax over 3
experts) and embed (region[32] @ W[32,2048] for the selected expert, + sinusoid
positional encoding).

Key formulation: for each expert, (unfold -> repeat_interleave -> linear) is a
single linear map region[32] -> out[2048].  We stack the 3 expert maps plus 16
positional-encoding rows (selected by a constant one-hot region-index
indicator) into one W_full [112, 2048].  On device the one-hot gates scale X
per-partition before a single K=112 matmul per 512-chunk, so expert selection
AND the PE add happen inside the PE array.
"""

import math
import sys

import numpy as np

for _p in ("/opt/trn_rl_repo",):
    if _p not in sys.path:
        sys.path.insert(0, _p)

import concourse.bass as bass
import concourse.mybir as mybir
import concourse.tile as tile
from concourse import bacc
from concourse.bass_utils import run_bass_kernel_spmd

F32 = mybir.dt.float32
F32R = mybir.dt.float32r

B, C, L = 32, 64, 512
D_MODEL = 512
MAX_P = 32
TAU = 0.5
PATCH_LENS = (8, 16, 32)
BC = B * C                  # 2048
R = L // MAX_P              # 16
T = MAX_P // min(PATCH_LENS)  # 4
N_REG = BC * R              # 32768 regions
N_CORES = 8
REG_PER_CORE = N_REG // N_CORES  # 4096
TILE_P = 128
N_TILES = REG_PER_CORE // TILE_P  # 32
KA = 3 * MAX_P + R          # 112 augmented contraction
OUT_W = T * D_MODEL         # 2048
N_CHUNK = 512
N_CHUNKS = OUT_W // N_CHUNK  # 4


def _sinusoid_pe_np(num_pos, d_model):
    pos = np.arange(num_pos, dtype=np.float32)[:, None]
    div = np.exp(
        np.arange(0, d_model, 2, dtype=np.float32)
        * np.float32(-(math.log(10000.0) / d_model))
    )
    pe = np.zeros((num_pos, d_model), dtype=np.float32)
    pe[:, 0::2] = np.sin(pos * div)
    pe[:, 1::2] = np.cos(pos * div)
    return pe


def _build_wfull(emb_w8, emb_w16, emb_w32):
    w = np.zeros((KA, OUT_W), dtype=np.float32)
    for e, (p, we) in enumerate(zip(PATCH_LENS, (emb_w8, emb_w16, emb_w32))):
        n_p = MAX_P // p
        rep = T - n_p + 1
        for t in range(T):
            s = min(t // rep, n_p - 1)
            w[e * MAX_P + p * s : e * MAX_P + p * s + p, t * D_MODEL : (t + 1) * D_MODEL] += we
    pe = _sinusoid_pe_np(R * T, D_MODEL).reshape(R, OUT_W)
    w[3 * MAX_P :, :] = pe
    return w


_CACHED = {}


def _build_nc(big_dtype, repeat=1):
    nc = bacc.Bacc("TRN2", target_bir_lowering=False, debug=False)
    x_in = nc.dram_tensor("x_in", [REG_PER_CORE, MAX_P], F32, kind="ExternalInput").ap()
    g_in = nc.dram_tensor("g_in", [REG_PER_CORE, 3], F32, kind="ExternalInput").ap()
    wf_in = nc.dram_tensor("wf_in", [KA, OUT_W], big_dtype, kind="ExternalInput").ap()
    w1_in = nc.dram_tensor("w1_in", [MAX_P + 1, 64], F32, kind="ExternalInput").ap()
    w2_in = nc.dram_tensor("w2_in", [65, 3], F32, kind="ExternalInput").ap()
    id_in = nc.dram_tensor("id_in", [128, 128], F32, kind="ExternalInput").ap()
    i16_in = nc.dram_tensor("i16_in", [R, 128], big_dtype, kind="ExternalInput").ap()
    out_d = nc.dram_tensor("out", [REG_PER_CORE, OUT_W], F32, kind="ExternalOutput").ap()
    idx_d = nc.dram_tensor("idx_out", [N_TILES, TILE_P], F32, kind="ExternalOutput").ap()

    RELU = mybir.ActivationFunctionType.Relu
    AX = mybir.AxisListType.X
    OP = mybir.AluOpType

    with tile.TileContext(nc) as tc:
        with (
            tc.tile_pool(name="const", bufs=1) as constp,
            tc.tile_pool(name="xin", bufs=4) as xinp,
            tc.tile_pool(name="small", bufs=3) as smallp,
            tc.tile_pool(name="xa", bufs=3) as xap,
            tc.tile_pool(name="xat", bufs=3) as xatp,
            tc.tile_pool(name="outs", bufs=3) as outsp,
            tc.tile_pool(name="ps", bufs=3, space="PSUM") as psp,
            tc.tile_pool(name="pout", bufs=4, space="PSUM") as poutp,
        ):
            # ---- constants, loaded once ----
            wf_s = constp.tile([KA, OUT_W], big_dtype)
            nc.gpsimd.dma_start(wf_s[:], wf_in[:])
            w1_s = constp.tile([MAX_P + 1, 64], F32)
            nc.gpsimd.dma_start(w1_s[:], w1_in[:])
            w2_s = constp.tile([65, 3], F32)
            nc.gpsimd.dma_start(w2_s[:], w2_in[:])
            id_s = constp.tile([128, 128], F32)
            nc.gpsimd.dma_start(id_s[:], id_in[:])
            i16_s = constp.tile([R, 128], big_dtype)
            nc.gpsimd.dma_start(i16_s[:], i16_in[:])
            gmb_s = constp.tile([TILE_P, N_TILES, 3], F32)
            nc.gpsimd.dma_start(
                gmb_s[:], g_in.rearrange("(t p) k -> p t k", p=TILE_P)
            )
            idx_acc = constp.tile([TILE_P, N_TILES], F32)

            import contextlib

            rep_ctx = (
                tc.For_i(0, repeat, 1) if repeat > 1 else contextlib.nullcontext()
            )
            with rep_ctx:
                _tile_body(nc, tc, locals())

            # ---- write idx (transposed so DMA is contiguous) ----
            ti = psp.tile([N_TILES, TILE_P], F32, tag="ps")
            nc.tensor.transpose(ti[:], idx_acc[:], id_s[:])
            ti_s = smallp.tile([N_TILES, TILE_P], F32, tag="tis")
            nc.scalar.copy(ti_s[:], ti[:])
            nc.sync.dma_start(idx_d[:], ti_s[:])

    nc.compile()
    return nc


def _tile_body(nc, tc, env):
    RELU = mybir.ActivationFunctionType.Relu
    AX = mybir.AxisListType.X
    OP = mybir.AluOpType
    xinp, smallp, xap, xatp, outsp, psp, poutp = (
        env["xinp"], env["smallp"], env["xap"], env["xatp"], env["outsp"],
        env["psp"], env["poutp"],
    )
    x_in, out_d = env["x_in"], env["out_d"]
    wf_s, w1_s, w2_s, id_s, i16_s, gmb_s, idx_acc = (
        env["wf_s"], env["w1_s"], env["w2_s"], env["id_s"], env["i16_s"],
        env["gmb_s"], env["idx_acc"],
    )
    big_dtype = env["big_dtype"]
    if True:
            for t in range(N_TILES):
                # ---- load X tile ----
                xt = xinp.tile([TILE_P, MAX_P], F32)
                nc.gpsimd.dma_start(xt[:], x_in[bass.ts(t, TILE_P), :])

                # ---- router ----
                t1 = psp.tile([MAX_P, TILE_P], F32, tag="ps")
                nc.tensor.transpose(t1[:], xt[:], id_s[:])
                xt_s = smallp.tile([MAX_P + 1, TILE_P], F32, tag="xts")
                nc.scalar.copy(xt_s[0:MAX_P, :], t1[:])
                nc.gpsimd.memset(xt_s[MAX_P : MAX_P + 1, :], 1.0)

                h_ps = psp.tile([TILE_P, 64], F32, tag="ps")
                nc.tensor.matmul(h_ps[:], xt_s[:], w1_s[:])
                h_s = smallp.tile([TILE_P, 64], F32, tag="hs")
                nc.scalar.activation(h_s[:], h_ps[:], RELU)

                t2 = psp.tile([64, TILE_P], F32, tag="ps")
                nc.tensor.transpose(t2[:], h_s[:], id_s[:])
                ht_s = smallp.tile([65, TILE_P], F32, tag="hts")
                nc.scalar.copy(ht_s[0:64, :], t2[:])
                nc.gpsimd.memset(ht_s[64:65, :], 1.0)

                lg_ps = psp.tile([TILE_P, 3], F32, tag="ps")
                nc.tensor.matmul(lg_ps[:], ht_s[:], w2_s[:])

                # ---- argmax + one-hot gates (no softmax needed) ----
                z = smallp.tile([TILE_P, 3], F32, tag="z")
                nc.vector.tensor_add(z[:], lg_ps[:], gmb_s[:, t, :])
                m = smallp.tile([TILE_P, 1], F32, tag="m")
                nc.vector.tensor_reduce(m[:], z[:], axis=AX, op=OP.max)
                e0 = smallp.tile([TILE_P, 1], F32, tag="e0")
                nc.vector.tensor_tensor(e0[:], z[:, 0:1], m[:], op=OP.is_equal)
                e1 = smallp.tile([TILE_P, 1], F32, tag="e1")
                nc.vector.tensor_tensor(e1[:], z[:, 1:2], m[:], op=OP.is_equal)
                a = smallp.tile([TILE_P, 1], F32, tag="a")
                nc.vector.tensor_scalar(a[:], e0[:], 0.0, None, op0=OP.is_equal)
                g1 = smallp.tile([TILE_P, 1], F32, tag="g1")
                nc.vector.tensor_tensor(g1[:], a[:], e1[:], op=OP.mult)
                g2 = smallp.tile([TILE_P, 1], F32, tag="g2")
                nc.vector.tensor_tensor(g2[:], a[:], g1[:], op=OP.subtract)
                bb = smallp.tile([TILE_P, 1], F32, tag="bb")
                nc.vector.tensor_scalar(
                    bb[:], e1[:], -1.0, 2.0, op0=OP.mult, op1=OP.add
                )
                nc.vector.tensor_tensor(
                    idx_acc[:, t : t + 1], a[:], bb[:], op=OP.mult
                )

                # ---- gated, augmented X ----
                xa = xap.tile([TILE_P, 3 * MAX_P], F32)
                nc.vector.tensor_scalar(xa[:, 0:32], xt[:], e0[:], None, op0=OP.mult)
                nc.vector.tensor_scalar(xa[:, 32:64], xt[:], g1[:], None, op0=OP.mult)
                nc.vector.tensor_scalar(xa[:, 64:96], xt[:], g2[:], None, op0=OP.mult)

                t3 = psp.tile([3 * MAX_P, TILE_P], F32, tag="ps")
                nc.tensor.transpose(t3[:], xa[:], id_s[:])
                xat = xatp.tile([KA, TILE_P], big_dtype)
                nc.scalar.copy(xat[0 : 3 * MAX_P, :], t3[:])
                nc.vector.tensor_copy(xat[3 * MAX_P : KA, :], i16_s[:])

                # ---- fused expert+PE matmul, 4 chunks of 512 ----
                out_s = outsp.tile([TILE_P, OUT_W], F32)
                for c in range(N_CHUNKS):
                    ops = poutp.tile([TILE_P, N_CHUNK], F32, tag="pout")
                    nc.tensor.matmul(ops[:], xat[:], wf_s[:, bass.ts(c, N_CHUNK)])
                    nc.vector.tensor_copy(out_s[:, bass.ts(c, N_CHUNK)], ops[:])
                nc.sync.dma_start(out_d[bass.ts(t, TILE_P), :], out_s[:])


def _prep_inputs(x, gumbel, cls_w1, cls_b1, cls_w2, cls_b2, emb_w8, emb_w16, emb_w32,
                 big_np):
    x2 = np.ascontiguousarray(x.reshape(N_REG, MAX_P), dtype=np.float32)
    g2 = np.ascontiguousarray(gumbel.reshape(N_REG, 3), dtype=np.float32)
    wfull = _build_wfull(
        np.asarray(emb_w8, np.float32),
        np.asarray(emb_w16, np.float32),
        np.asarray(emb_w32, np.float32),
    ).astype(big_np)
    w1a = np.concatenate(
        [np.asarray(cls_w1, np.float32), np.asarray(cls_b1, np.float32)[None, :]], 0
    )
    w2a = np.concatenate(
        [np.asarray(cls_w2, np.float32), np.asarray(cls_b2, np.float32)[None, :]], 0
    )
    ident = np.eye(128, dtype=np.float32)
    i16t = np.tile(np.eye(R, dtype=np.float32), (1, TILE_P // R)).reshape(R, TILE_P)
    # i16t[r, p] must be 1 iff p % 16 == r
    i16t = np.zeros((R, TILE_P), dtype=np.float32)
    i16t[np.arange(TILE_P) % R, np.arange(TILE_P)] = 1.0
    i16t = i16t.astype(big_np)

    in_maps = []
    for c in range(N_CORES):
        sl = slice(c * REG_PER_CORE, (c + 1) * REG_PER_CORE)
        in_maps.append(
            {
                "x_in": x2[sl],
                "g_in": g2[sl],
                "wf_in": wfull,
                "w1_in": w1a,
                "w2_in": w2a,
                "id_in": ident,
                "i16_in": i16t,
            }
        )
    return in_maps


def _run(inputs, trace=False, big="f32"):
    big_dtype = {"f32": F32, "f32r": F32R, "bf16": mybir.dt.bfloat16}[big]
    big_np = {"f32": np.float32, "f32r": np.float32, "bf16": None}[big]
    if big == "bf16":
        import ml_dtypes

        big_np = ml_dtypes.bfloat16
    key = big
    if key not in _CACHED:
        _CACHED[key] = _build_nc(big_dtype)
    nc = _CACHED[key]
    in_maps = _prep_inputs(**inputs, big_np=big_np)
    res = run_bass_kernel_spmd(nc, in_maps, core_ids=list(range(N_CORES)), trace=trace)
    outs = res.results
    x_patch = np.concatenate([outs[c]["out"] for c in range(N_CORES)], axis=0)
    x_patch = x_patch.reshape(BC, R * T, D_MODEL)
    idx = np.concatenate(
        [outs[c]["idx_out"].reshape(-1) for c in range(N_CORES)], axis=0
    )
    all_cls_pred = idx.reshape(BC, R).T.reshape(-1).astype(np.int32)
    return (x_patch, all_cls_pred), res


def kernel(**inputs):
    (x_patch, all_cls_pred), _ = _run(inputs, trace=False)
    return x_patch, all_cls_pred


if __name__ == "__main__":
    rng = np.random.default_rng(0)
    fake = {
        "x": rng.standard_normal((B, C, L), dtype=np.float32),
        "gumbel": rng.random((BC, R, 3), dtype=np.float32),
        "cls_w1": rng.standard_normal((MAX_P, 64), dtype=np.float32) * 0.1,
        "cls_b1": rng.standard_normal((64,), dtype=np.float32) * 0.1,
        "cls_w2": rng.standard_normal((64, 3), dtype=np.float32) * 0.1,
        "cls_b2": rng.standard_normal((3,), dtype=np.float32) * 0.1,
        "emb_w8": rng.standard_normal((8, D_MODEL), dtype=np.float32) * 0.3,
        "emb_w16": rng.standard_normal((16, D_MODEL), dtype=np.float32) * 0.2,
        "emb_w32": rng.standard_normal((32, D_MODEL), dtype=np.float32) * 0.17,
    }
    out, _ = _run(fake, trace=False)
    print("ok", out[0].shape, out[1].shape, out[0].dtype, out[1].dtype)
